# revision 22
# baseline (speedup 1.0000x reference)
"""Block-sparse flash attention (local + vertical-stride pattern) on 8 TRN2
NeuronCores.

Sharding: tensor-parallel over heads. Core c gets q-heads [4c, 4c+4) and
kv-head c (the GQA group maps exactly: q-head h uses kv-head h//4). No
collectives needed; outputs are concatenated along the feature dim on host.

v2 pipeline notes (why it's structured this way): the scalar engine's exp
is the hard roofline for this problem (~48us of ACT streaming per core at
1 elem/lane/cycle), so the whole schedule is built to keep ACT saturated:

  - Scores are computed transposed, S^T[kv, q], per 128-wide kv chunk, at
    live causal-prefix widths, exactly as before (the slot plan per tile
    is unchanged).
  - exp runs over GROUPS of whole slots greedily packed up to 1536 f32
    (3 PSUM banks), SPANNING tile boundaries: ~10 ACT instructions per
    head instead of 13, with P^T for a whole head written into one
    contiguous SBUF buffer so a group's single activation can cross tile
    edges. Dependency tracking is range-based, so ACT writing group g+1
    never falsifies against PV reading tile t's columns.
  - Emission discipline per group: [score MMs of g] -> [ACT g] ->
    [masks of tiles completed by g] -> [PV + norm backlog of tiles
    completed by g-1]. Score matmuls always sit at the front of the PE
    queue so the next exp's inputs are ready while the current exp runs;
    PV fills the PE's ACT-shadow; masks are emitted on the vector queue
    BEFORE norms so a norm waiting on PV can never block the masks the
    next PV needs.
  - The staging-slot selector is folded into the prefix mask (one
    host-built [128, 512] mask applied with a 2-region strided AP), so
    each t>=4 tile needs 2 vector ops instead of 3.
  - Input DMAs are issued from the (otherwise idle) GpSimd queue - each
    dma_start costs ~650ns of queue time and the Sync queue was the
    reason inputs took 25us to land in the old version. First pieces are
    small so the first matmuls start as early as possible.
  - 8 dummy N=512 matmuls on a zeroed tile run during the DMA window to
    flip the PE HAM clock-gate to 8/8 before the real matmuls arrive.
  - PV accumulates O[q, d] with lhsT=P^T chunk and rhs=[V | 1]; the ones
    column makes the softmax denominator fall out of the same matmuls.
    The last head runs tiles descending so the pipeline drains on the
    smallest tile.
"""

import numpy as np
import ml_dtypes

BF16 = ml_dtypes.bfloat16

# Problem constants (hardcoded; see module docstring).
S = 2048
NUM_HEADS = 32
NUM_KV_HEADS = 8
D = 128
BLK = 64
LOCAL_BLOCKS = 16
VSTRIDE = 8
SCALE = 0.08838834764831845
NCORES = 8
HPC = NUM_HEADS // NCORES          # heads per core = 4
QTILE = 256                        # q rows per tile (4 sparse blocks)
NT = S // QTILE                    # 8 tiles
NCHUNK = S // 128                  # 16 kv chunks of 128
GROUP_W = 1024                     # exp group budget (2 PSUM banks of f32)
S_TOT = 14336                      # total score columns per head


def _tile_plan(j, t):
    """Static slot plan for head-slot j (0..3), q-tile t. Core-independent.

    Returns a list of slots (kind, aux, col, width):
      kind "loc"  - fully-live local 128-kv chunk; aux = first block of pair
      kind "pfx0" - leading chunk (lo, lo+1), live q-prefix [0, 64)
      kind "pfx1" - leading chunk (lo+2, lo+3), live q-prefix [0, 192)
      kind "stg"  - host-staged remote chunk (2 block slots); aux = stage idx
      kind "d1"   - diagonal half chunk (4t+2, 4t+3), q cols [128, 256)
    The per-core selection of remote blocks lives entirely in host data
    (KS/VS/PMASK), so the program is identical on all 8 cores.
    """
    slots = []
    col = 0

    def add(kind, aux, w):
        nonlocal col
        slots.append((kind, aux, col, w))
        col += w

    if t < 4:
        for i in range(2 * t + 1):
            add("loc", 2 * i, QTILE)
        add("d1", 4 * t + 2, 128)
    else:
        lo = 4 * t - 16
        add("pfx0", lo, 64)
        add("pfx1", lo + 2, 192)
        for i in range(6):
            add("loc", lo + 4 + 2 * i, QTILE)
        add("stg", j * 4 + (t - 4), QTILE)
        add("loc", 4 * t, QTILE)          # D0 diagonal chunk
        add("d1", 4 * t + 2, 128)
    return slots


def _remote_class(core, j):
    """Blocks b with b % 8 == this value are remote-visible for head 4*core+j."""
    return (-(4 * core + j + 1)) % VSTRIDE


def _stage_blocks(core, j, t):
    """Remote blocks host-staged for (core, head-slot j, tile t>=4): all
    vertical-stride-selected blocks at or below the local window's leading
    staircase (b < 4t-12), at most 2."""
    r = _remote_class(core, j)
    return [b for b in range(4 * t - 12) if b % VSTRIDE == r]


def _head_schedule(j, tile_order, solo_tiles=()):
    """Greedy-pack the head's slot stream (tiles in processing order) into
    exp groups of whole slots with width <= GROUP_W. Tiles in solo_tiles
    get their own group boundary (used to keep head 0's first exps small
    so compute starts before the bulk of the input DMA lands).

    Returns (tiles_meta, groups):
      tiles_meta[t] = {"base": global col of tile start, "slots": [...]}
      groups[k] = {"base", "width", "pieces": [(t, kind, aux, gcol, w)],
                   "completes": [t, ...]}  # tiles whose last slot is here
    """
    tiles_meta = {}
    groups = []
    cur = None
    gcol = 0
    for t in tile_order:
        slots = _tile_plan(j, t)
        tiles_meta[t] = {"base": gcol, "slots": slots}
        for (kind, aux, col, w) in slots:
            if cur is not None and cur["width"] + w > GROUP_W:
                groups.append(cur)
                cur = None
            if cur is None:
                cur = {"base": gcol, "width": 0, "pieces": [], "completes": []}
            cur["pieces"].append((t, kind, aux, gcol, w))
            cur["width"] += w
            gcol += w
        cur["completes"].append(t)
        if t in solo_tiles:
            groups.append(cur)
            cur = None
    if cur is not None and cur["width"]:
        groups.append(cur)
    assert gcol == S_TOT
    return tiles_meta, groups


_CACHE = {}


def _build_nc():
    import concourse.bacc as bacc
    import concourse.tile as tile
    from concourse import mybir

    dt = mybir.dt
    nc = bacc.Bacc(None)

    qT = nc.declare_dram_parameter("qT", [HPC * D, S], dt.bfloat16, isOutput=False)
    kT = nc.declare_dram_parameter("kT", [D, S], dt.bfloat16, isOutput=False)
    v1 = nc.declare_dram_parameter("v1", [D, NCHUNK * 129], dt.bfloat16, isOutput=False)
    m1x = nc.declare_dram_parameter("m1x", [D, 384], dt.bfloat16, isOutput=False)
    pmask = nc.declare_dram_parameter("pmask", [D, HPC * 4 * 512], dt.bfloat16, isOutput=False)
    ks = nc.declare_dram_parameter("ks", [D, HPC * 4 * 128], dt.bfloat16, isOutput=False)
    vs = nc.declare_dram_parameter("vs", [D, HPC * 4 * 129], dt.bfloat16, isOutput=False)
    out = nc.declare_dram_parameter("out", [S, HPC * D], dt.float32, isOutput=True)

    EXP = mybir.ActivationFunctionType.Exp

    with tile.TileContext(nc) as tc:
        with (
            tc.tile_pool(name="consts", bufs=1) as consts,
            tc.tile_pool(name="ptp", bufs=2) as ptp,
            tc.tile_pool(name="ohp", bufs=2) as ohp,
            tc.tile_pool(name="lp", bufs=4) as lp,
            tc.tile_pool(name="stp", bufs=3, space="PSUM") as stp,
            tc.tile_pool(name="opp", bufs=2, space="PSUM") as opp,
        ):
            # warm the ACT exp table while input DMAs are in flight
            DUMI = consts.tile([128, 1], dt.float32, tag="dumi")
            DUMO = consts.tile([128, 1], dt.bfloat16, tag="dumo")
            nc.vector.memset(DUMI, 0.0)
            nc.scalar.activation(DUMO, DUMI, EXP, scale=1.0)

            # HAM warm-up: dummy matmuls keep the PE busy while the input
            # DMA lands so the clock-gate is 8/8 when real matmuls start
            DUM2 = consts.tile([128, 256], dt.bfloat16, tag="dum2")
            nc.vector.memset(DUM2, 0.0)
            ST_warm = stp.tile([128, GROUP_W], dt.float32, tag="st")
            for _ in range(6):
                nc.tensor.matmul(ST_warm[:, 0:256], lhsT=DUM2[:, 0:128],
                                 rhs=DUM2, start=True, stop=True)


            # Input DMAs issue from the GpSimd queue (idle otherwise); each
            # dma_start costs ~650ns of issue time on its queue, so putting
            # ~12 of them on Sync would gate the input pipeline. First
            # pieces are small so tile (h0,t0..t2) can start ASAP.
            KT = consts.tile([128, S], dt.bfloat16, tag="kt")
            QT = [consts.tile([128, S], dt.bfloat16, name=f"qt{h}", tag=f"qt{h}") for h in range(HPC)]
            V1 = consts.tile([128, NCHUNK * 129], dt.bfloat16, tag="v1")
            M1X = consts.tile([128, 384], dt.bfloat16, tag="m1x")
            # pieces sized so each group's inputs complete (sem fires per
            # dma_start, so a too-big piece gates early groups on its
            # tail), alternated between the otherwise-idle GpSimd and
            # Vector queues - each dma_start costs ~650ns of issue time,
            # and a single queue serializes the whole input pipeline
            KS = consts.tile([128, HPC * 4 * 128], dt.bfloat16, tag="ks")
            VS = consts.tile([128, HPC * 4 * 129], dt.bfloat16, tag="vs")
            PMASK = consts.tile([128, HPC * 4 * 512], dt.bfloat16, tag="pmask")
            nc.gpsimd.dma_start(out=KT[:, 0:256], in_=kT[:, 0:256])
            nc.gpsimd.dma_start(out=QT[0][:, 0:256], in_=qT[0:128, 0:256])
            nc.gpsimd.dma_start(out=KT[:, 256:512], in_=kT[:, 256:512])
            nc.gpsimd.dma_start(out=QT[0][:, 256:512], in_=qT[0:128, 256:512])
            nc.gpsimd.dma_start(out=M1X, in_=m1x[:, :])
            nc.gpsimd.dma_start(out=V1[:, 0:4 * 129], in_=v1[:, 0:4 * 129])
            nc.gpsimd.dma_start(out=KT[:, 512:1024], in_=kT[:, 512:1024])
            nc.gpsimd.dma_start(out=QT[0][:, 512:1024], in_=qT[0:128, 512:1024])
            nc.gpsimd.dma_start(out=KS, in_=ks[:, :])
            nc.gpsimd.dma_start(out=VS, in_=vs[:, :])
            nc.gpsimd.dma_start(out=KT[:, 1024:S], in_=kT[:, 1024:S])
            nc.gpsimd.dma_start(out=QT[0][:, 1024:S], in_=qT[0:128, 1024:S])
            nc.gpsimd.dma_start(out=V1[:, 4 * 129:], in_=v1[:, 4 * 129:])
            nc.gpsimd.dma_start(out=PMASK, in_=pmask[:, :])
            for h in range(1, HPC):
                nc.gpsimd.dma_start(out=QT[h], in_=qT[h * 128:(h + 1) * 128, :])

            out_r = out.rearrange("(c p) m -> p c m", p=128)  # [128, 16, 512]

            def emit_group_mms(h, grp, ST):
                """Score matmuls for one exp group, split at PSUM bank
                boundaries (a matmul output cannot cross a 512-f32 bank).
                Wide streams first so each LDWEIGHTS hides under the
                previous matmul's rhs stream."""
                base = grp["base"]
                subs = []
                for (t, kind, aux, gcol, w) in grp["pieces"]:
                    pcol = gcol - base
                    qs = QT[h][:, t * QTILE:(t + 1) * QTILE]
                    qstart = 128 if kind == "d1" else 0
                    if kind == "stg":
                        lhsT = KS[:, aux * 128: aux * 128 + 128]
                    else:
                        lhsT = KT[:, aux * BLK: aux * BLK + 128]
                    s0 = pcol
                    while s0 < pcol + w:
                        s1 = min(pcol + w, (s0 // 512 + 1) * 512)
                        subs.append((s1 - s0, s0, lhsT,
                                     qs[:, qstart + (s0 - pcol): qstart + (s1 - pcol)]))
                        s0 = s1
                subs.sort(key=lambda x: -x[0])
                for w_, off, lhsT, rr in subs:
                    nc.tensor.matmul(ST[:, off:off + w_], lhsT=lhsT, rhs=rr,
                                     start=True, stop=True)

            def emit_pmask(h, t, tiles_meta, PT):
                # one 2-region op: prefix staircase mask (cols 0:256 of
                # the tile) + staging-slot selector (cols 1792:2048),
                # via a stride-1792 AP; host merged both into PMASK.
                # Emitted as soon as the group covering tile col 2048 has
                # been exp'd (1-2 groups before the tile completes), so it
                # is off the PV critical path.
                tb = tiles_meta[t]["base"]
                idx = h * 4 + (t - 4)
                ap = PT[:, tb:tb + 2048].rearrange(
                    "p (r c) -> p r c", c=256)[:, 0:8:7, :]
                nc.vector.tensor_mul(
                    ap, ap,
                    PMASK[:, idx * 512:(idx + 1) * 512].rearrange(
                        "p (r c) -> p r c", c=256))

            def emit_m1x(h, t, tiles_meta, PT):
                # combined causal mask over the adjacent D0+D1 diagonal
                # chunks (M1X = [tri(256) | tri(128)], 384 wide)
                tb = tiles_meta[t]["base"]
                d0 = 2048 if t >= 4 else 512 * t
                nc.vector.tensor_mul(PT[:, tb + d0: tb + d0 + 384],
                                     PT[:, tb + d0: tb + d0 + 384], M1X)

            def pv_half(h, t, tiles_meta, PT, OP, s):
                tb = tiles_meta[t]["base"]
                slots = tiles_meta[t]["slots"]
                # full-partition mms first so start=True covers all 128 rows;
                # the partial prefix mms (64 live q rows) accumulate after.
                fulls, partials = [], []
                for kind, aux, scol, w in slots:
                    col = tb + scol
                    if kind == "loc":
                        fulls.append((PT[:, col + s * 128: col + s * 128 + 128], 128,
                                      V1[:, (aux // 2) * 129: (aux // 2) * 129 + 129]))
                    elif kind == "stg":
                        fulls.append((PT[:, col + s * 128: col + s * 128 + 128], 128,
                                      VS[:, aux * 129: aux * 129 + 129]))
                    elif kind == "d1":
                        if s == 1:
                            fulls.append((PT[:, col: col + 128], 128,
                                          V1[:, (aux // 2) * 129: (aux // 2) * 129 + 129]))
                    elif kind == "pfx0":
                        if s == 0:
                            partials.append((PT[:, col: col + 64], 64,
                                             V1[:, (aux // 2) * 129: (aux // 2) * 129 + 129]))
                    elif kind == "pfx1":
                        v = V1[:, (aux // 2) * 129: (aux // 2) * 129 + 129]
                        if s == 0:
                            fulls.append((PT[:, col: col + 128], 128, v))
                        else:
                            partials.append((PT[:, col + 128: col + 192], 64, v))
                n = len(fulls) + len(partials)
                for mm, (lhsT, qw, rhs) in enumerate(fulls + partials):
                    o_ap = OP[:, s * 129:(s + 1) * 129] if qw == 128 \
                        else OP[0:qw, s * 129:(s + 1) * 129]
                    nc.tensor.matmul(o_ap, lhsT=lhsT, rhs=rhs,
                                     start=(mm == 0), stop=(mm == n - 1))

            def pv_norm(h, t, OP, OH):
                # normalize: O / L (L = ones-column at col 128 of each half)
                LI = lp.tile([128, 2], dt.float32, tag="li")
                l_ap = OP[:, :].rearrange("p (s x) -> p s x", s=2)[:, :, 128]
                nc.vector.reciprocal(LI, l_ap)
                for s in range(2):
                    nc.vector.tensor_scalar_mul(
                        OH[:, 2 * t + s, :],
                        OP[:, s * 129: s * 129 + 128],
                        LI[:, s:s + 1],
                    )
                if h < HPC - 1:
                    store = {3: (0, 8), 7: (8, 16)}.get(t)
                else:
                    # descending head: upper half mid-stream, then two
                    # small pieces so the final DMA stays short
                    store = {4: (8, 16), 2: (4, 8), 0: (0, 4)}.get(t)
                if store is not None:
                    c0, c1 = store
                    nc.sync.dma_start(out=out_r[:, c0:c1, h * 128:(h + 1) * 128],
                                      in_=OH[:, c0:c1, :])

            # ---- the weave ------------------------------------------------
            # Per group (iteration): [score MMs of g] -> [ACT g] -> [masks
            # whose exp'd columns this group completed] -> [paced PV
            # flush]. PV halves drain from a global backlog, paced by
            # virtual engine clocks so emitted PE work tracks emitted ACT
            # work: heavy tiles' PV spills into the light-tile iterations
            # where ACT is the slower engine. The natural ~2-group PV lag
            # keeps masks and OP-pool reuse off the PV critical path, and
            # the score burst at the iteration head keeps ACT fed.
            from collections import deque
            pvq = deque()      # pending units: (gk, h, t, tiles_meta, PT, OH, s)
            op_open = {}       # (h, t) -> OP tile in flight
            gk = 0             # global group counter

            def flush_one_half():
                (_g, hh, tt, tm, pt_, oh_, s) = pvq.popleft()
                if s == 0:
                    OP = opp.tile([128, 2 * 129], dt.float32, tag="op")
                    op_open[(hh, tt)] = OP
                    pv_half(hh, tt, tm, pt_, OP, 0)
                elif s == 1:
                    pv_half(hh, tt, tm, pt_, op_open[(hh, tt)], 1)
                else:
                    pv_norm(hh, tt, op_open.pop((hh, tt)), oh_)
            for h in range(HPC):
                order = list(range(NT)) if h < HPC - 1 else list(range(NT - 1, -1, -1))
                tiles_meta, groups = _head_schedule(
                    h, order, solo_tiles=(0, 1) if h == 0 else ())
                # attach each mask to the first group covering its columns
                mask_after = [[] for _ in groups]
                ends = []
                acc = 0
                for grp in groups:
                    acc += grp["width"]
                    ends.append(acc)
                base0 = tiles_meta[order[0]]["base"]
                for t in order:
                    tb = tiles_meta[t]["base"]
                    tw = sum(w for _, _, _, w in tiles_meta[t]["slots"])
                    if t >= 4:
                        gp = next(i for i, e in enumerate(ends) if e >= tb + 2048)
                        mask_after[gp].append(("pmask", t))
                    ge = next(i for i, e in enumerate(ends) if e >= tb + tw)
                    mask_after[ge].append(("m1x", t))
                PT = ptp.tile([128, S_TOT], dt.bfloat16, tag="pt")
                OH = ohp.tile([128, NCHUNK, 128], dt.float32, tag="oh")
                for gi, grp in enumerate(groups):
                    ST = stp.tile([128, GROUP_W], dt.float32, tag="st")
                    emit_group_mms(h, grp, ST)
                    nc.scalar.activation(
                        PT[:, grp["base"]: grp["base"] + grp["width"]],
                        ST[:, :grp["width"]], EXP, scale=SCALE)
                    for kind, t in mask_after[gi]:
                        if kind == "pmask":
                            emit_pmask(h, t, tiles_meta, PT)
                        else:
                            emit_m1x(h, t, tiles_meta, PT)
                    # drain up to 3 backlog units (pv half / norm) per
                    # group, but only units enqueued >= 2 groups ago: the
                    # masks a PV half reads must be ~2 ACT periods cold or
                    # its weight-loads stall the whole in-order PE queue
                    for _ in range(3):
                        if pvq and pvq[0][0] <= gk - 2:
                            flush_one_half()
                    for t in grp["completes"]:
                        pvq.append((gk, h, t, tiles_meta, PT, OH, 0))
                        pvq.append((gk, h, t, tiles_meta, PT, OH, 1))
                        pvq.append((gk, h, t, tiles_meta, PT, OH, 2))
                    gk += 1
            while pvq:
                flush_one_half()

    nc.finalize()
    return nc


def _host_inputs(query, key, value):
    """Build the 8 per-core input maps (host-side sharding + layout prep)."""
    q = np.asarray(query, dtype=np.float32)
    k = np.asarray(key, dtype=np.float32)
    v = np.asarray(value, dtype=np.float32)

    pp = np.arange(128)[:, None]
    qq = np.arange(QTILE)[None, :]
    m1x = np.concatenate([(qq >= pp).astype(np.float32),
                          (qq[:, :128] >= pp).astype(np.float32)],
                         axis=1).astype(BF16)              # [128, 384]

    in_maps = []
    for c in range(NCORES):
        qTc = np.ascontiguousarray(q[:, c * 512:(c + 1) * 512].T).astype(BF16)
        kTc = np.ascontiguousarray(k[:, c * D:(c + 1) * D].T).astype(BF16)
        vc = v[:, c * D:(c + 1) * D]                         # [2048, 128]
        vch = vc.reshape(NCHUNK, 128, D).transpose(1, 0, 2)  # [128, 16, 128]
        v1c = np.ones((128, NCHUNK, 129), dtype=np.float32)
        v1c[:, :, :128] = vch
        v1c = v1c.reshape(128, NCHUNK * 129).astype(BF16)

        # staging: per (head-slot j, tile t>=4) up to 2 vertical-stride
        # blocks packed into one 128-row chunk; dead slots zero + masked
        ksc = np.zeros((128, HPC, 4, 128), dtype=np.float32)
        vsc = np.ones((128, HPC, 4, 129), dtype=np.float32)
        vsc[:, :, :, :128] = 0.0
        # pmask[..., 0:256]: leading staircase mask (pfx0 cols 0:64 =
        # blocks lo/lo+1 at q prefix 64; pfx1 cols 64:256 = blocks
        # lo+2/lo+3 at prefix 192), remote-selected blocks excluded
        # (staged instead). pmask[..., 256:512]: staging-slot selector
        # (per-partition, broadcast over the 256 staging columns).
        pmaskc = np.zeros((128, HPC, 4, 512), dtype=np.float32)
        for j in range(HPC):
            r = _remote_class(c, j)
            for t in range(4, 8):
                lo = 4 * t - 16
                for si, b in enumerate(_stage_blocks(c, j, t)):
                    ksc[:, j, t - 4, si * 64:(si + 1) * 64] = kTc[:, b * BLK:(b + 1) * BLK]
                    vsc[si * 64:(si + 1) * 64, j, t - 4, :128] = vch[
                        (b % 2) * 64:(b % 2) * 64 + 64, b // 2, :]
                    pmaskc[si * 64:(si + 1) * 64, j, t - 4, 256:512] = 1.0
                if (lo + 1) % VSTRIDE != r:
                    pmaskc[64:128, j, t - 4, 0:64] = 1.0
                if (lo + 2) % VSTRIDE != r:
                    pmaskc[0:64, j, t - 4, 64:192] = 1.0
                if (lo + 3) % VSTRIDE != r:
                    pmaskc[64:128, j, t - 4, 64:256] = 1.0

        in_maps.append({
            "qT": qTc,
            "kT": kTc,
            "v1": v1c,
            "m1x": m1x,
            "pmask": pmaskc.reshape(128, HPC * 4 * 512).astype(BF16),
            "ks": ksc.reshape(128, HPC * 4 * 128).astype(BF16),
            "vs": vsc.reshape(128, HPC * 4 * 129).astype(BF16),
        })
    return in_maps


def _get_nc():
    if "nc" not in _CACHE:
        _CACHE["nc"] = _build_nc()
    return _CACHE["nc"]


def kernel(query, key, value):
    from concourse.bass_utils import run_bass_kernel_spmd

    nc = _get_nc()
    in_maps = _host_inputs(query, key, value)
    res = run_bass_kernel_spmd(nc, in_maps, core_ids=list(range(NCORES)))
    outs = [res.results[c]["out"] for c in range(NCORES)]
    return np.concatenate(outs, axis=1).astype(np.float32)


if __name__ == "__main__":
    rng = np.random.default_rng(0)
    q = rng.standard_normal((S, NUM_HEADS * D), dtype=np.float32)
    k = rng.standard_normal((S, NUM_KV_HEADS * D), dtype=np.float32)
    v = rng.standard_normal((S, NUM_KV_HEADS * D), dtype=np.float32)
    o = kernel(query=q, key=k, value=v)
    print("kernel output", o.shape, o.dtype, np.abs(o).max())


# revision 23
# speedup vs baseline: 1.0486x; 1.0486x over previous
"""Block-sparse flash attention (local + vertical-stride pattern) on 8 TRN2
NeuronCores.

Sharding: tensor-parallel over heads. Core c gets q-heads [4c, 4c+4) and
kv-head c (the GQA group maps exactly: q-head h uses kv-head h//4). No
collectives needed; outputs are concatenated along the feature dim on host.

v2 pipeline notes (why it's structured this way): the scalar engine's exp
is the hard roofline for this problem (~48us of ACT streaming per core at
1 elem/lane/cycle), so the whole schedule is built to keep ACT saturated:

  - Scores are computed transposed, S^T[kv, q], per 128-wide kv chunk, at
    live causal-prefix widths, exactly as before (the slot plan per tile
    is unchanged).
  - exp runs over GROUPS of whole slots greedily packed up to 1536 f32
    (3 PSUM banks), SPANNING tile boundaries: ~10 ACT instructions per
    head instead of 13, with P^T for a whole head written into one
    contiguous SBUF buffer so a group's single activation can cross tile
    edges. Dependency tracking is range-based, so ACT writing group g+1
    never falsifies against PV reading tile t's columns.
  - Emission discipline per group: [score MMs of g] -> [ACT g] ->
    [masks of tiles completed by g] -> [PV + norm backlog of tiles
    completed by g-1]. Score matmuls always sit at the front of the PE
    queue so the next exp's inputs are ready while the current exp runs;
    PV fills the PE's ACT-shadow; masks are emitted on the vector queue
    BEFORE norms so a norm waiting on PV can never block the masks the
    next PV needs.
  - The staging-slot selector is folded into the prefix mask (one
    host-built [128, 512] mask applied with a 2-region strided AP), so
    each t>=4 tile needs 2 vector ops instead of 3.
  - Input DMAs are issued from the (otherwise idle) GpSimd queue - each
    dma_start costs ~650ns of queue time and the Sync queue was the
    reason inputs took 25us to land in the old version. First pieces are
    small so the first matmuls start as early as possible.
  - 8 dummy N=512 matmuls on a zeroed tile run during the DMA window to
    flip the PE HAM clock-gate to 8/8 before the real matmuls arrive.
  - PV accumulates O[q, d] with lhsT=P^T chunk and rhs=[V | 1]; the ones
    column makes the softmax denominator fall out of the same matmuls.
    The last head runs tiles descending so the pipeline drains on the
    smallest tile.
"""

import numpy as np
import ml_dtypes

BF16 = ml_dtypes.bfloat16

# Problem constants (hardcoded; see module docstring).
S = 2048
NUM_HEADS = 32
NUM_KV_HEADS = 8
D = 128
BLK = 64
LOCAL_BLOCKS = 16
VSTRIDE = 8
SCALE = 0.08838834764831845
NCORES = 8
HPC = NUM_HEADS // NCORES          # heads per core = 4
QTILE = 256                        # q rows per tile (4 sparse blocks)
NT = S // QTILE                    # 8 tiles
NCHUNK = S // 128                  # 16 kv chunks of 128
GROUP_W = 1536                     # exp group budget (3 PSUM banks of f32)
S_TOT = 14336                      # total score columns per head


def _tile_plan(j, t):
    """Static slot plan for head-slot j (0..3), q-tile t. Core-independent.

    Returns a list of slots (kind, aux, col, width):
      kind "loc"  - fully-live local 128-kv chunk; aux = first block of pair
      kind "pfx0" - leading chunk (lo, lo+1), live q-prefix [0, 64)
      kind "pfx1" - leading chunk (lo+2, lo+3), live q-prefix [0, 192)
      kind "stg"  - host-staged remote chunk (2 block slots); aux = stage idx
      kind "d1"   - diagonal half chunk (4t+2, 4t+3), q cols [128, 256)
    The per-core selection of remote blocks lives entirely in host data
    (KS/VS/PMASK), so the program is identical on all 8 cores.
    """
    slots = []
    col = 0

    def add(kind, aux, w):
        nonlocal col
        slots.append((kind, aux, col, w))
        col += w

    if t < 4:
        for i in range(2 * t + 1):
            add("loc", 2 * i, QTILE)
        add("d1", 4 * t + 2, 128)
    else:
        lo = 4 * t - 16
        add("pfx0", lo, 64)
        add("pfx1", lo + 2, 192)
        for i in range(6):
            add("loc", lo + 4 + 2 * i, QTILE)
        add("stg", j * 4 + (t - 4), QTILE)
        add("loc", 4 * t, QTILE)          # D0 diagonal chunk
        add("d1", 4 * t + 2, 128)
    return slots


def _remote_class(core, j):
    """Blocks b with b % 8 == this value are remote-visible for head 4*core+j."""
    return (-(4 * core + j + 1)) % VSTRIDE


def _stage_blocks(core, j, t):
    """Remote blocks host-staged for (core, head-slot j, tile t>=4): all
    vertical-stride-selected blocks at or below the local window's leading
    staircase (b < 4t-12), at most 2."""
    r = _remote_class(core, j)
    return [b for b in range(4 * t - 12) if b % VSTRIDE == r]


def _head_schedule(j, tile_order, solo_tiles=()):
    """Greedy-pack the head's slot stream (tiles in processing order) into
    exp groups of whole slots with width <= GROUP_W. Tiles in solo_tiles
    get their own group boundary (used to keep head 0's first exps small
    so compute starts before the bulk of the input DMA lands).

    Returns (tiles_meta, groups):
      tiles_meta[t] = {"base": global col of tile start, "slots": [...]}
      groups[k] = {"base", "width", "pieces": [(t, kind, aux, gcol, w)],
                   "completes": [t, ...]}  # tiles whose last slot is here
    """
    tiles_meta = {}
    groups = []
    cur = None
    gcol = 0
    for t in tile_order:
        slots = _tile_plan(j, t)
        tiles_meta[t] = {"base": gcol, "slots": slots}
        for (kind, aux, col, w) in slots:
            if cur is not None and cur["width"] + w > GROUP_W:
                groups.append(cur)
                cur = None
            if cur is None:
                cur = {"base": gcol, "width": 0, "pieces": [], "completes": []}
            cur["pieces"].append((t, kind, aux, gcol, w))
            cur["width"] += w
            gcol += w
        cur["completes"].append(t)
        if t in solo_tiles:
            groups.append(cur)
            cur = None
    if cur is not None and cur["width"]:
        groups.append(cur)
    assert gcol == S_TOT
    return tiles_meta, groups


_CACHE = {}


def _build_nc():
    import concourse.bacc as bacc
    import concourse.tile as tile
    from concourse import mybir

    dt = mybir.dt
    nc = bacc.Bacc(None)

    qT = nc.declare_dram_parameter("qT", [HPC * D, S], dt.bfloat16, isOutput=False)
    kT = nc.declare_dram_parameter("kT", [D, S], dt.bfloat16, isOutput=False)
    v1 = nc.declare_dram_parameter("v1", [D, NCHUNK * 129], dt.bfloat16, isOutput=False)
    m1x = nc.declare_dram_parameter("m1x", [D, 384], dt.bfloat16, isOutput=False)
    pmask = nc.declare_dram_parameter("pmask", [D, HPC * 4 * 512], dt.bfloat16, isOutput=False)
    ks = nc.declare_dram_parameter("ks", [D, HPC * 4 * 128], dt.bfloat16, isOutput=False)
    vs = nc.declare_dram_parameter("vs", [D, HPC * 4 * 129], dt.bfloat16, isOutput=False)
    out = nc.declare_dram_parameter("out", [S, HPC * D], dt.float32, isOutput=True)

    EXP = mybir.ActivationFunctionType.Exp

    with tile.TileContext(nc) as tc:
        with (
            tc.tile_pool(name="consts", bufs=1) as consts,
            tc.tile_pool(name="ptp", bufs=2) as ptp,
            tc.tile_pool(name="ohp", bufs=3) as ohp,
            tc.tile_pool(name="lp", bufs=4) as lp,
            tc.tile_pool(name="stp", bufs=2, space="PSUM") as stp,
            tc.tile_pool(name="opp", bufs=2, space="PSUM") as opp,
        ):
            # warm the ACT exp table while input DMAs are in flight
            DUMI = consts.tile([128, 1], dt.float32, tag="dumi")
            DUMO = consts.tile([128, 1], dt.bfloat16, tag="dumo")
            nc.vector.memset(DUMI, 0.0)
            nc.scalar.activation(DUMO, DUMI, EXP, scale=1.0)

            # HAM warm-up: dummy matmuls keep the PE busy while the input
            # DMA lands so the clock-gate is 8/8 when real matmuls start
            DUM2 = consts.tile([128, 256], dt.bfloat16, tag="dum2")
            nc.vector.memset(DUM2, 0.0)
            ST_warm = stp.tile([128, GROUP_W], dt.float32, tag="st")
            for _ in range(6):
                nc.tensor.matmul(ST_warm[:, 0:256], lhsT=DUM2[:, 0:128],
                                 rhs=DUM2, start=True, stop=True)


            # Input DMAs issue from the GpSimd queue (idle otherwise); each
            # dma_start costs ~650ns of issue time on its queue, so putting
            # ~12 of them on Sync would gate the input pipeline. First
            # pieces are small so tile (h0,t0..t2) can start ASAP.
            KT = consts.tile([128, S], dt.bfloat16, tag="kt")
            QT = [consts.tile([128, S], dt.bfloat16, name=f"qt{h}", tag=f"qt{h}") for h in range(HPC)]
            V1 = consts.tile([128, NCHUNK * 129], dt.bfloat16, tag="v1")
            M1X = consts.tile([128, 384], dt.bfloat16, tag="m1x")
            # pieces sized so each group's inputs complete (sem fires per
            # dma_start, so a too-big piece gates early groups on its
            # tail), alternated between the otherwise-idle GpSimd and
            # Vector queues - each dma_start costs ~650ns of issue time,
            # and a single queue serializes the whole input pipeline
            KS = consts.tile([128, HPC * 4 * 128], dt.bfloat16, tag="ks")
            VS = consts.tile([128, HPC * 4 * 129], dt.bfloat16, tag="vs")
            PMASK = consts.tile([128, HPC * 4 * 512], dt.bfloat16, tag="pmask")
            nc.gpsimd.dma_start(out=KT[:, 0:256], in_=kT[:, 0:256])
            nc.gpsimd.dma_start(out=QT[0][:, 0:256], in_=qT[0:128, 0:256])
            nc.gpsimd.dma_start(out=KT[:, 256:512], in_=kT[:, 256:512])
            nc.gpsimd.dma_start(out=QT[0][:, 256:512], in_=qT[0:128, 256:512])
            nc.gpsimd.dma_start(out=M1X, in_=m1x[:, :])
            nc.gpsimd.dma_start(out=V1[:, 0:4 * 129], in_=v1[:, 0:4 * 129])
            nc.gpsimd.dma_start(out=KT[:, 512:1024], in_=kT[:, 512:1024])
            nc.gpsimd.dma_start(out=QT[0][:, 512:1024], in_=qT[0:128, 512:1024])
            nc.gpsimd.dma_start(out=KS, in_=ks[:, :])
            nc.gpsimd.dma_start(out=VS, in_=vs[:, :])
            nc.gpsimd.dma_start(out=KT[:, 1024:S], in_=kT[:, 1024:S])
            nc.gpsimd.dma_start(out=QT[0][:, 1024:S], in_=qT[0:128, 1024:S])
            nc.gpsimd.dma_start(out=V1[:, 4 * 129:], in_=v1[:, 4 * 129:])
            nc.gpsimd.dma_start(out=PMASK, in_=pmask[:, :])
            for h in range(1, HPC):
                nc.gpsimd.dma_start(out=QT[h], in_=qT[h * 128:(h + 1) * 128, :])

            out_r = out.rearrange("(c p) m -> p c m", p=128)  # [128, 16, 512]

            def emit_group_mms(h, grp, ST):
                """Score matmuls for one exp group, split at PSUM bank
                boundaries (a matmul output cannot cross a 512-f32 bank).
                Wide streams first so each LDWEIGHTS hides under the
                previous matmul's rhs stream."""
                base = grp["base"]
                subs = []
                for (t, kind, aux, gcol, w) in grp["pieces"]:
                    pcol = gcol - base
                    qs = QT[h][:, t * QTILE:(t + 1) * QTILE]
                    qstart = 128 if kind == "d1" else 0
                    if kind == "stg":
                        lhsT = KS[:, aux * 128: aux * 128 + 128]
                    else:
                        lhsT = KT[:, aux * BLK: aux * BLK + 128]
                    s0 = pcol
                    while s0 < pcol + w:
                        s1 = min(pcol + w, (s0 // 512 + 1) * 512)
                        subs.append((s1 - s0, s0, lhsT,
                                     qs[:, qstart + (s0 - pcol): qstart + (s1 - pcol)]))
                        s0 = s1
                subs.sort(key=lambda x: -x[0])
                for w_, off, lhsT, rr in subs:
                    nc.tensor.matmul(ST[:, off:off + w_], lhsT=lhsT, rhs=rr,
                                     start=True, stop=True)

            def emit_pmask(h, t, tiles_meta, PT):
                # one 2-region op: prefix staircase mask (cols 0:256 of
                # the tile) + staging-slot selector (cols 1792:2048),
                # via a stride-1792 AP; host merged both into PMASK.
                # Emitted as soon as the group covering tile col 2048 has
                # been exp'd (1-2 groups before the tile completes), so it
                # is off the PV critical path.
                tb = tiles_meta[t]["base"]
                idx = h * 4 + (t - 4)
                ap = PT[:, tb:tb + 2048].rearrange(
                    "p (r c) -> p r c", c=256)[:, 0:8:7, :]
                nc.vector.tensor_mul(
                    ap, ap,
                    PMASK[:, idx * 512:(idx + 1) * 512].rearrange(
                        "p (r c) -> p r c", c=256))

            def emit_m1x(h, t, tiles_meta, PT):
                # combined causal mask over the adjacent D0+D1 diagonal
                # chunks (M1X = [tri(256) | tri(128)], 384 wide)
                tb = tiles_meta[t]["base"]
                d0 = 2048 if t >= 4 else 512 * t
                nc.vector.tensor_mul(PT[:, tb + d0: tb + d0 + 384],
                                     PT[:, tb + d0: tb + d0 + 384], M1X)

            def pv_half(h, t, tiles_meta, PT, OP, s):
                tb = tiles_meta[t]["base"]
                slots = tiles_meta[t]["slots"]
                # full-partition mms first so start=True covers all 128 rows;
                # the partial prefix mms (64 live q rows) accumulate after.
                fulls, partials = [], []
                for kind, aux, scol, w in slots:
                    col = tb + scol
                    if kind == "loc":
                        fulls.append((PT[:, col + s * 128: col + s * 128 + 128], 128,
                                      V1[:, (aux // 2) * 129: (aux // 2) * 129 + 129]))
                    elif kind == "stg":
                        fulls.append((PT[:, col + s * 128: col + s * 128 + 128], 128,
                                      VS[:, aux * 129: aux * 129 + 129]))
                    elif kind == "d1":
                        if s == 1:
                            fulls.append((PT[:, col: col + 128], 128,
                                          V1[:, (aux // 2) * 129: (aux // 2) * 129 + 129]))
                    elif kind == "pfx0":
                        if s == 0:
                            partials.append((PT[:, col: col + 64], 64,
                                             V1[:, (aux // 2) * 129: (aux // 2) * 129 + 129]))
                    elif kind == "pfx1":
                        v = V1[:, (aux // 2) * 129: (aux // 2) * 129 + 129]
                        if s == 0:
                            fulls.append((PT[:, col: col + 128], 128, v))
                        else:
                            partials.append((PT[:, col + 128: col + 192], 64, v))
                n = len(fulls) + len(partials)
                for mm, (lhsT, qw, rhs) in enumerate(fulls + partials):
                    o_ap = OP[:, s * 129:(s + 1) * 129] if qw == 128 \
                        else OP[0:qw, s * 129:(s + 1) * 129]
                    nc.tensor.matmul(o_ap, lhsT=lhsT, rhs=rhs,
                                     start=(mm == 0), stop=(mm == n - 1))

            def pv_norm(h, t, OP, OH):
                # normalize: O / L (L = ones-column at col 128 of each half)
                LI = lp.tile([128, 2], dt.float32, tag="li")
                l_ap = OP[:, :].rearrange("p (s x) -> p s x", s=2)[:, :, 128]
                nc.vector.reciprocal(LI, l_ap)
                for s in range(2):
                    nc.vector.tensor_scalar_mul(
                        OH[:, 2 * t + s, :],
                        OP[:, s * 129: s * 129 + 128],
                        LI[:, s:s + 1],
                    )
                if h < HPC - 1:
                    store = {3: (0, 8), 7: (8, 16)}.get(t)
                else:
                    # descending head: upper half mid-stream, then two
                    # small pieces so the final DMA stays short
                    store = {4: (8, 16), 2: (4, 8), 0: (0, 4)}.get(t)
                if store is not None:
                    c0, c1 = store
                    nc.sync.dma_start(out=out_r[:, c0:c1, h * 128:(h + 1) * 128],
                                      in_=OH[:, c0:c1, :])

            # ---- the weave ------------------------------------------------
            # Per group (iteration): [score MMs of g] -> [ACT g] -> [masks
            # whose exp'd columns this group completed] -> [paced PV
            # flush]. PV halves drain from a global backlog, paced by
            # virtual engine clocks so emitted PE work tracks emitted ACT
            # work: heavy tiles' PV spills into the light-tile iterations
            # where ACT is the slower engine. The natural ~2-group PV lag
            # keeps masks and OP-pool reuse off the PV critical path, and
            # the score burst at the iteration head keeps ACT fed.
            from collections import deque
            pvq = deque()      # pending units: (gk, h, t, tiles_meta, PT, OH, s)
            op_open = {}       # (h, t) -> OP tile in flight
            gk = 0             # global group counter

            def flush_one_half():
                (_g, hh, tt, tm, pt_, oh_, s) = pvq.popleft()
                if s == 0:
                    OP = opp.tile([128, 2 * 129], dt.float32, tag="op")
                    op_open[(hh, tt)] = OP
                    pv_half(hh, tt, tm, pt_, OP, 0)
                elif s == 1:
                    pv_half(hh, tt, tm, pt_, op_open[(hh, tt)], 1)
                else:
                    pv_norm(hh, tt, op_open.pop((hh, tt)), oh_)
            for h in range(HPC):
                order = list(range(NT)) if h < HPC - 1 else list(range(NT - 1, -1, -1))
                tiles_meta, groups = _head_schedule(
                    h, order, solo_tiles=(0, 1) if h == 0 else ())
                # attach each mask to the first group covering its columns
                mask_after = [[] for _ in groups]
                ends = []
                acc = 0
                for grp in groups:
                    acc += grp["width"]
                    ends.append(acc)
                base0 = tiles_meta[order[0]]["base"]
                for t in order:
                    tb = tiles_meta[t]["base"]
                    tw = sum(w for _, _, _, w in tiles_meta[t]["slots"])
                    if t >= 4:
                        gp = next(i for i, e in enumerate(ends) if e >= tb + 2048)
                        mask_after[gp].append(("pmask", t))
                    ge = next(i for i, e in enumerate(ends) if e >= tb + tw)
                    mask_after[ge].append(("m1x", t))
                PT = ptp.tile([128, S_TOT], dt.bfloat16, tag="pt")
                OH = ohp.tile([128, NCHUNK, 128], dt.float32, tag="oh")
                for gi, grp in enumerate(groups):
                    ST = stp.tile([128, GROUP_W], dt.float32, tag="st")
                    emit_group_mms(h, grp, ST)
                    nc.scalar.activation(
                        PT[:, grp["base"]: grp["base"] + grp["width"]],
                        ST[:, :grp["width"]], EXP, scale=SCALE)
                    for kind, t in mask_after[gi]:
                        if kind == "pmask":
                            emit_pmask(h, t, tiles_meta, PT)
                        else:
                            emit_m1x(h, t, tiles_meta, PT)
                    # drain up to 3 backlog units (pv half / norm) per
                    # group, but only units enqueued >= 2 groups ago: the
                    # masks a PV half reads must be ~2 ACT periods cold or
                    # its weight-loads stall the whole in-order PE queue
                    for _ in range(3):
                        if pvq and pvq[0][0] <= gk - 2:
                            flush_one_half()
                    for t in grp["completes"]:
                        pvq.append((gk, h, t, tiles_meta, PT, OH, 0))
                        pvq.append((gk, h, t, tiles_meta, PT, OH, 1))
                        pvq.append((gk, h, t, tiles_meta, PT, OH, 2))
                    gk += 1
            while pvq:
                flush_one_half()

    nc.finalize()
    return nc


def _host_inputs(query, key, value):
    """Build the 8 per-core input maps (host-side sharding + layout prep)."""
    q = np.asarray(query, dtype=np.float32)
    k = np.asarray(key, dtype=np.float32)
    v = np.asarray(value, dtype=np.float32)

    pp = np.arange(128)[:, None]
    qq = np.arange(QTILE)[None, :]
    m1x = np.concatenate([(qq >= pp).astype(np.float32),
                          (qq[:, :128] >= pp).astype(np.float32)],
                         axis=1).astype(BF16)              # [128, 384]

    in_maps = []
    for c in range(NCORES):
        qTc = np.ascontiguousarray(q[:, c * 512:(c + 1) * 512].T).astype(BF16)
        kTc = np.ascontiguousarray(k[:, c * D:(c + 1) * D].T).astype(BF16)
        vc = v[:, c * D:(c + 1) * D]                         # [2048, 128]
        vch = vc.reshape(NCHUNK, 128, D).transpose(1, 0, 2)  # [128, 16, 128]
        v1c = np.ones((128, NCHUNK, 129), dtype=np.float32)
        v1c[:, :, :128] = vch
        v1c = v1c.reshape(128, NCHUNK * 129).astype(BF16)

        # staging: per (head-slot j, tile t>=4) up to 2 vertical-stride
        # blocks packed into one 128-row chunk; dead slots zero + masked
        ksc = np.zeros((128, HPC, 4, 128), dtype=np.float32)
        vsc = np.ones((128, HPC, 4, 129), dtype=np.float32)
        vsc[:, :, :, :128] = 0.0
        # pmask[..., 0:256]: leading staircase mask (pfx0 cols 0:64 =
        # blocks lo/lo+1 at q prefix 64; pfx1 cols 64:256 = blocks
        # lo+2/lo+3 at prefix 192), remote-selected blocks excluded
        # (staged instead). pmask[..., 256:512]: staging-slot selector
        # (per-partition, broadcast over the 256 staging columns).
        pmaskc = np.zeros((128, HPC, 4, 512), dtype=np.float32)
        for j in range(HPC):
            r = _remote_class(c, j)
            for t in range(4, 8):
                lo = 4 * t - 16
                for si, b in enumerate(_stage_blocks(c, j, t)):
                    ksc[:, j, t - 4, si * 64:(si + 1) * 64] = kTc[:, b * BLK:(b + 1) * BLK]
                    vsc[si * 64:(si + 1) * 64, j, t - 4, :128] = vch[
                        (b % 2) * 64:(b % 2) * 64 + 64, b // 2, :]
                    pmaskc[si * 64:(si + 1) * 64, j, t - 4, 256:512] = 1.0
                if (lo + 1) % VSTRIDE != r:
                    pmaskc[64:128, j, t - 4, 0:64] = 1.0
                if (lo + 2) % VSTRIDE != r:
                    pmaskc[0:64, j, t - 4, 64:192] = 1.0
                if (lo + 3) % VSTRIDE != r:
                    pmaskc[64:128, j, t - 4, 64:256] = 1.0

        in_maps.append({
            "qT": qTc,
            "kT": kTc,
            "v1": v1c,
            "m1x": m1x,
            "pmask": pmaskc.reshape(128, HPC * 4 * 512).astype(BF16),
            "ks": ksc.reshape(128, HPC * 4 * 128).astype(BF16),
            "vs": vsc.reshape(128, HPC * 4 * 129).astype(BF16),
        })
    return in_maps


def _get_nc():
    if "nc" not in _CACHE:
        _CACHE["nc"] = _build_nc()
    return _CACHE["nc"]


def kernel(query, key, value):
    from concourse.bass_utils import run_bass_kernel_spmd

    nc = _get_nc()
    in_maps = _host_inputs(query, key, value)
    res = run_bass_kernel_spmd(nc, in_maps, core_ids=list(range(NCORES)))
    outs = [res.results[c]["out"] for c in range(NCORES)]
    return np.concatenate(outs, axis=1).astype(np.float32)


if __name__ == "__main__":
    rng = np.random.default_rng(0)
    q = rng.standard_normal((S, NUM_HEADS * D), dtype=np.float32)
    k = rng.standard_normal((S, NUM_KV_HEADS * D), dtype=np.float32)
    v = rng.standard_normal((S, NUM_KV_HEADS * D), dtype=np.float32)
    o = kernel(query=q, key=k, value=v)
    print("kernel output", o.shape, o.dtype, np.abs(o).max())


# revision 24
# speedup vs baseline: 1.0647x; 1.0154x over previous
"""Block-sparse flash attention (local + vertical-stride pattern) on 8 TRN2
NeuronCores.

Sharding: tensor-parallel over heads. Core c gets q-heads [4c, 4c+4) and
kv-head c (the GQA group maps exactly: q-head h uses kv-head h//4). No
collectives needed; outputs are concatenated along the feature dim on host.

v2 pipeline notes (why it's structured this way): the scalar engine's exp
is the hard roofline for this problem (~48us of ACT streaming per core at
1 elem/lane/cycle), so the whole schedule is built to keep ACT saturated:

  - Scores are computed transposed, S^T[kv, q], per 128-wide kv chunk, at
    live causal-prefix widths, exactly as before (the slot plan per tile
    is unchanged).
  - exp runs over GROUPS of whole slots greedily packed up to 1536 f32
    (3 PSUM banks), SPANNING tile boundaries: ~10 ACT instructions per
    head instead of 13, with P^T for a whole head written into one
    contiguous SBUF buffer so a group's single activation can cross tile
    edges. Dependency tracking is range-based, so ACT writing group g+1
    never falsifies against PV reading tile t's columns.
  - Emission discipline per group: [score MMs of g] -> [ACT g] ->
    [masks of tiles completed by g] -> [PV + norm backlog of tiles
    completed by g-1]. Score matmuls always sit at the front of the PE
    queue so the next exp's inputs are ready while the current exp runs;
    PV fills the PE's ACT-shadow; masks are emitted on the vector queue
    BEFORE norms so a norm waiting on PV can never block the masks the
    next PV needs.
  - The staging-slot selector is folded into the prefix mask (one
    host-built [128, 512] mask applied with a 2-region strided AP), so
    each t>=4 tile needs 2 vector ops instead of 3.
  - Input DMAs are issued from the (otherwise idle) GpSimd queue - each
    dma_start costs ~650ns of queue time and the Sync queue was the
    reason inputs took 25us to land in the old version. First pieces are
    small so the first matmuls start as early as possible.
  - 8 dummy N=512 matmuls on a zeroed tile run during the DMA window to
    flip the PE HAM clock-gate to 8/8 before the real matmuls arrive.
  - PV accumulates O[q, d] with lhsT=P^T chunk and rhs=[V | 1]; the ones
    column makes the softmax denominator fall out of the same matmuls.
    The last head runs tiles descending so the pipeline drains on the
    smallest tile.
"""

import numpy as np
import ml_dtypes

BF16 = ml_dtypes.bfloat16

# Problem constants (hardcoded; see module docstring).
S = 2048
NUM_HEADS = 32
NUM_KV_HEADS = 8
D = 128
BLK = 64
LOCAL_BLOCKS = 16
VSTRIDE = 8
SCALE = 0.08838834764831845
NCORES = 8
HPC = NUM_HEADS // NCORES          # heads per core = 4
QTILE = 256                        # q rows per tile (4 sparse blocks)
NT = S // QTILE                    # 8 tiles
NCHUNK = S // 128                  # 16 kv chunks of 128
GROUP_W = 1536                     # exp group budget (3 PSUM banks of f32)
S_TOT = 14336                      # total score columns per head


def _tile_plan(j, t):
    """Static slot plan for head-slot j (0..3), q-tile t. Core-independent.

    Returns a list of slots (kind, aux, col, width):
      kind "loc"  - fully-live local 128-kv chunk; aux = first block of pair
      kind "pfx0" - leading chunk (lo, lo+1), live q-prefix [0, 64)
      kind "pfx1" - leading chunk (lo+2, lo+3), live q-prefix [0, 192)
      kind "stg"  - host-staged remote chunk (2 block slots); aux = stage idx
      kind "d1"   - diagonal half chunk (4t+2, 4t+3), q cols [128, 256)
    The per-core selection of remote blocks lives entirely in host data
    (KS/VS/PMASK), so the program is identical on all 8 cores.
    """
    slots = []
    col = 0

    def add(kind, aux, w):
        nonlocal col
        slots.append((kind, aux, col, w))
        col += w

    if t < 4:
        for i in range(2 * t + 1):
            add("loc", 2 * i, QTILE)
        add("d1", 4 * t + 2, 128)
    else:
        lo = 4 * t - 16
        add("pfx0", lo, 64)
        add("pfx1", lo + 2, 192)
        for i in range(6):
            add("loc", lo + 4 + 2 * i, QTILE)
        add("stg", j * 4 + (t - 4), QTILE)
        add("loc", 4 * t, QTILE)          # D0 diagonal chunk
        add("d1", 4 * t + 2, 128)
    return slots


def _remote_class(core, j):
    """Blocks b with b % 8 == this value are remote-visible for head 4*core+j."""
    return (-(4 * core + j + 1)) % VSTRIDE


def _stage_blocks(core, j, t):
    """Remote blocks host-staged for (core, head-slot j, tile t>=4): all
    vertical-stride-selected blocks at or below the local window's leading
    staircase (b < 4t-12), at most 2."""
    r = _remote_class(core, j)
    return [b for b in range(4 * t - 12) if b % VSTRIDE == r]


def _head_schedule(j, tile_order, solo_tiles=()):
    """Greedy-pack the head's slot stream (tiles in processing order) into
    exp groups of whole slots with width <= GROUP_W. Tiles in solo_tiles
    get their own group boundary (used to keep head 0's first exps small
    so compute starts before the bulk of the input DMA lands).

    Returns (tiles_meta, groups):
      tiles_meta[t] = {"base": global col of tile start, "slots": [...]}
      groups[k] = {"base", "width", "pieces": [(t, kind, aux, gcol, w)],
                   "completes": [t, ...]}  # tiles whose last slot is here
    """
    tiles_meta = {}
    groups = []
    cur = None
    gcol = 0
    for t in tile_order:
        slots = _tile_plan(j, t)
        tiles_meta[t] = {"base": gcol, "slots": slots}
        for (kind, aux, col, w) in slots:
            if cur is not None and cur["width"] + w > GROUP_W:
                groups.append(cur)
                cur = None
            if cur is None:
                cur = {"base": gcol, "width": 0, "pieces": [], "completes": []}
            cur["pieces"].append((t, kind, aux, gcol, w))
            cur["width"] += w
            gcol += w
        cur["completes"].append(t)
        if t in solo_tiles:
            groups.append(cur)
            cur = None
    if cur is not None and cur["width"]:
        groups.append(cur)
    assert gcol == S_TOT
    return tiles_meta, groups


_CACHE = {}


def _build_nc():
    import concourse.bacc as bacc
    import concourse.tile as tile
    from concourse import mybir

    dt = mybir.dt
    nc = bacc.Bacc(None)

    qT = nc.declare_dram_parameter("qT", [HPC * D, S], dt.bfloat16, isOutput=False)
    kT = nc.declare_dram_parameter("kT", [D, S], dt.bfloat16, isOutput=False)
    v1 = nc.declare_dram_parameter("v1", [D, NCHUNK * 129], dt.bfloat16, isOutput=False)
    m1x = nc.declare_dram_parameter("m1x", [D, 384], dt.bfloat16, isOutput=False)
    pmask = nc.declare_dram_parameter("pmask", [D, HPC * 4 * 512], dt.bfloat16, isOutput=False)
    ks = nc.declare_dram_parameter("ks", [D, HPC * 4 * 128], dt.bfloat16, isOutput=False)
    vs = nc.declare_dram_parameter("vs", [D, HPC * 4 * 129], dt.bfloat16, isOutput=False)
    out = nc.declare_dram_parameter("out", [S, HPC * D], dt.float32, isOutput=True)

    EXP = mybir.ActivationFunctionType.Exp

    with tile.TileContext(nc) as tc:
        with (
            tc.tile_pool(name="consts", bufs=1) as consts,
            tc.tile_pool(name="ptp", bufs=2) as ptp,
            tc.tile_pool(name="ohp", bufs=2) as ohp,
            tc.tile_pool(name="lp", bufs=4) as lp,
            tc.tile_pool(name="stp", bufs=2, space="PSUM") as stp,
            tc.tile_pool(name="opp", bufs=2, space="PSUM") as opp,
        ):
            # warm the ACT exp table while input DMAs are in flight
            DUMI = consts.tile([128, 1], dt.float32, tag="dumi")
            DUMO = consts.tile([128, 1], dt.bfloat16, tag="dumo")
            nc.vector.memset(DUMI, 0.0)
            nc.scalar.activation(DUMO, DUMI, EXP, scale=1.0)

            # HAM warm-up: dummy matmuls keep the PE busy while the input
            # DMA lands so the clock-gate is 8/8 when real matmuls start
            DUM2 = consts.tile([128, 512], dt.bfloat16, tag="dum2")
            nc.vector.memset(DUM2, 0.0)
            ST_warm = stp.tile([128, GROUP_W], dt.float32, tag="st")
            for _ in range(8):
                nc.tensor.matmul(ST_warm[:, 0:512], lhsT=DUM2[:, 0:128],
                                 rhs=DUM2, start=True, stop=True)


            # Input DMAs issue from the GpSimd queue (idle otherwise); each
            # dma_start costs ~650ns of issue time on its queue, so putting
            # ~12 of them on Sync would gate the input pipeline. First
            # pieces are small so tile (h0,t0..t2) can start ASAP.
            KT = consts.tile([128, S], dt.bfloat16, tag="kt")
            QT = [consts.tile([128, S], dt.bfloat16, name=f"qt{h}", tag=f"qt{h}") for h in range(HPC)]
            V1 = consts.tile([128, NCHUNK * 129], dt.bfloat16, tag="v1")
            M1X = consts.tile([128, 384], dt.bfloat16, tag="m1x")
            # pieces sized so each group's inputs complete (sem fires per
            # dma_start, so a too-big piece gates early groups on its
            # tail), alternated between the otherwise-idle GpSimd and
            # Vector queues - each dma_start costs ~650ns of issue time,
            # and a single queue serializes the whole input pipeline
            KS = consts.tile([128, HPC * 4 * 128], dt.bfloat16, tag="ks")
            VS = consts.tile([128, HPC * 4 * 129], dt.bfloat16, tag="vs")
            PMASK = consts.tile([128, HPC * 4 * 512], dt.bfloat16, tag="pmask")
            nc.gpsimd.dma_start(out=KT[:, 0:512], in_=kT[:, 0:512])
            nc.gpsimd.dma_start(out=QT[0][:, 0:512], in_=qT[0:128, 0:512])
            nc.gpsimd.dma_start(out=M1X, in_=m1x[:, :])
            nc.gpsimd.dma_start(out=V1[:, 0:4 * 129], in_=v1[:, 0:4 * 129])
            nc.gpsimd.dma_start(out=KT[:, 512:1024], in_=kT[:, 512:1024])
            nc.gpsimd.dma_start(out=QT[0][:, 512:1024], in_=qT[0:128, 512:1024])
            nc.gpsimd.dma_start(out=KS, in_=ks[:, :])
            nc.gpsimd.dma_start(out=VS, in_=vs[:, :])
            nc.gpsimd.dma_start(out=KT[:, 1024:S], in_=kT[:, 1024:S])
            nc.gpsimd.dma_start(out=QT[0][:, 1024:S], in_=qT[0:128, 1024:S])
            nc.gpsimd.dma_start(out=V1[:, 4 * 129:], in_=v1[:, 4 * 129:])
            nc.gpsimd.dma_start(out=PMASK, in_=pmask[:, :])
            for h in range(1, HPC):
                nc.gpsimd.dma_start(out=QT[h], in_=qT[h * 128:(h + 1) * 128, :])

            out_r = out.rearrange("(c p) m -> p c m", p=128)  # [128, 16, 512]

            def emit_group_mms(h, grp, ST):
                """Score matmuls for one exp group, split at PSUM bank
                boundaries (a matmul output cannot cross a 512-f32 bank).
                Wide streams first so each LDWEIGHTS hides under the
                previous matmul's rhs stream."""
                base = grp["base"]
                subs = []
                for (t, kind, aux, gcol, w) in grp["pieces"]:
                    pcol = gcol - base
                    qs = QT[h][:, t * QTILE:(t + 1) * QTILE]
                    qstart = 128 if kind == "d1" else 0
                    if kind == "stg":
                        lhsT = KS[:, aux * 128: aux * 128 + 128]
                    else:
                        lhsT = KT[:, aux * BLK: aux * BLK + 128]
                    s0 = pcol
                    while s0 < pcol + w:
                        s1 = min(pcol + w, (s0 // 512 + 1) * 512)
                        subs.append((s1 - s0, s0, lhsT,
                                     qs[:, qstart + (s0 - pcol): qstart + (s1 - pcol)]))
                        s0 = s1
                subs.sort(key=lambda x: -x[0])
                for w_, off, lhsT, rr in subs:
                    nc.tensor.matmul(ST[:, off:off + w_], lhsT=lhsT, rhs=rr,
                                     start=True, stop=True)

            def emit_pmask(h, t, tiles_meta, PT):
                # one 2-region op: prefix staircase mask (cols 0:256 of
                # the tile) + staging-slot selector (cols 1792:2048),
                # via a stride-1792 AP; host merged both into PMASK.
                # Emitted as soon as the group covering tile col 2048 has
                # been exp'd (1-2 groups before the tile completes), so it
                # is off the PV critical path.
                tb = tiles_meta[t]["base"]
                idx = h * 4 + (t - 4)
                ap = PT[:, tb:tb + 2048].rearrange(
                    "p (r c) -> p r c", c=256)[:, 0:8:7, :]
                nc.vector.tensor_mul(
                    ap, ap,
                    PMASK[:, idx * 512:(idx + 1) * 512].rearrange(
                        "p (r c) -> p r c", c=256))

            def emit_m1x(h, t, tiles_meta, PT):
                # combined causal mask over the adjacent D0+D1 diagonal
                # chunks (M1X = [tri(256) | tri(128)], 384 wide)
                tb = tiles_meta[t]["base"]
                d0 = 2048 if t >= 4 else 512 * t
                nc.vector.tensor_mul(PT[:, tb + d0: tb + d0 + 384],
                                     PT[:, tb + d0: tb + d0 + 384], M1X)

            def pv_half(h, t, tiles_meta, PT, OP, s):
                tb = tiles_meta[t]["base"]
                slots = tiles_meta[t]["slots"]
                # full-partition mms first so start=True covers all 128 rows;
                # the partial prefix mms (64 live q rows) accumulate after.
                fulls, partials = [], []
                for kind, aux, scol, w in slots:
                    col = tb + scol
                    if kind == "loc":
                        fulls.append((PT[:, col + s * 128: col + s * 128 + 128], 128,
                                      V1[:, (aux // 2) * 129: (aux // 2) * 129 + 129]))
                    elif kind == "stg":
                        fulls.append((PT[:, col + s * 128: col + s * 128 + 128], 128,
                                      VS[:, aux * 129: aux * 129 + 129]))
                    elif kind == "d1":
                        if s == 1:
                            fulls.append((PT[:, col: col + 128], 128,
                                          V1[:, (aux // 2) * 129: (aux // 2) * 129 + 129]))
                    elif kind == "pfx0":
                        if s == 0:
                            partials.append((PT[:, col: col + 64], 64,
                                             V1[:, (aux // 2) * 129: (aux // 2) * 129 + 129]))
                    elif kind == "pfx1":
                        v = V1[:, (aux // 2) * 129: (aux // 2) * 129 + 129]
                        if s == 0:
                            fulls.append((PT[:, col: col + 128], 128, v))
                        else:
                            partials.append((PT[:, col + 128: col + 192], 64, v))
                n = len(fulls) + len(partials)
                for mm, (lhsT, qw, rhs) in enumerate(fulls + partials):
                    o_ap = OP[:, s * 129:(s + 1) * 129] if qw == 128 \
                        else OP[0:qw, s * 129:(s + 1) * 129]
                    nc.tensor.matmul(o_ap, lhsT=lhsT, rhs=rhs,
                                     start=(mm == 0), stop=(mm == n - 1))

            def pv_norm(h, t, OP, OH):
                # normalize: O / L (L = ones-column at col 128 of each half)
                LI = lp.tile([128, 2], dt.float32, tag="li")
                l_ap = OP[:, :].rearrange("p (s x) -> p s x", s=2)[:, :, 128]
                nc.vector.reciprocal(LI, l_ap)
                for s in range(2):
                    nc.vector.tensor_scalar_mul(
                        OH[:, 2 * t + s, :],
                        OP[:, s * 129: s * 129 + 128],
                        LI[:, s:s + 1],
                    )
                if h < HPC - 1:
                    store = {3: (0, 8), 5: (8, 12), 7: (12, 16)}.get(t)
                else:
                    # descending head: store fine-grained so the final DMA
                    # after the last tile is only 2 chunks
                    store = {4: (8, 16), 3: (6, 8), 2: (4, 6),
                             1: (2, 4), 0: (0, 2)}.get(t)
                if store is not None:
                    c0, c1 = store
                    nc.sync.dma_start(out=out_r[:, c0:c1, h * 128:(h + 1) * 128],
                                      in_=OH[:, c0:c1, :])

            # ---- the weave ------------------------------------------------
            # Per group (iteration): [score MMs of g] -> [ACT g] -> [masks
            # whose exp'd columns this group completed] -> [paced PV
            # flush]. PV halves drain from a global backlog, paced by
            # virtual engine clocks so emitted PE work tracks emitted ACT
            # work: heavy tiles' PV spills into the light-tile iterations
            # where ACT is the slower engine. The natural ~2-group PV lag
            # keeps masks and OP-pool reuse off the PV critical path, and
            # the score burst at the iteration head keeps ACT fed.
            from collections import deque
            pvq = deque()      # pending units: (gk, h, t, tiles_meta, PT, OH, s)
            op_open = {}       # (h, t) -> OP tile in flight
            gk = 0             # global group counter

            def flush_one_half():
                (_g, hh, tt, tm, pt_, oh_, s) = pvq.popleft()
                if s == 0:
                    OP = opp.tile([128, 2 * 129], dt.float32, tag="op")
                    op_open[(hh, tt)] = OP
                    pv_half(hh, tt, tm, pt_, OP, 0)
                elif s == 1:
                    pv_half(hh, tt, tm, pt_, op_open[(hh, tt)], 1)
                else:
                    pv_norm(hh, tt, op_open.pop((hh, tt)), oh_)
            for h in range(HPC):
                order = list(range(NT)) if h < HPC - 1 else list(range(NT - 1, -1, -1))
                tiles_meta, groups = _head_schedule(h, order)
                # attach each mask to the first group covering its columns
                mask_after = [[] for _ in groups]
                ends = []
                acc = 0
                for grp in groups:
                    acc += grp["width"]
                    ends.append(acc)
                base0 = tiles_meta[order[0]]["base"]
                for t in order:
                    tb = tiles_meta[t]["base"]
                    tw = sum(w for _, _, _, w in tiles_meta[t]["slots"])
                    if t >= 4:
                        gp = next(i for i, e in enumerate(ends) if e >= tb + 2048)
                        mask_after[gp].append(("pmask", t))
                    ge = next(i for i, e in enumerate(ends) if e >= tb + tw)
                    mask_after[ge].append(("m1x", t))
                PT = ptp.tile([128, S_TOT], dt.bfloat16, tag="pt")
                OH = ohp.tile([128, NCHUNK, 128], dt.float32, tag="oh")
                for gi, grp in enumerate(groups):
                    ST = stp.tile([128, GROUP_W], dt.float32, tag="st")
                    emit_group_mms(h, grp, ST)
                    nc.scalar.activation(
                        PT[:, grp["base"]: grp["base"] + grp["width"]],
                        ST[:, :grp["width"]], EXP, scale=SCALE)
                    for kind, t in mask_after[gi]:
                        if kind == "pmask":
                            emit_pmask(h, t, tiles_meta, PT)
                        else:
                            emit_m1x(h, t, tiles_meta, PT)
                    # drain up to 3 backlog units (pv half / norm) per
                    # group, but only units enqueued >= 2 groups ago: the
                    # masks a PV half reads must be ~2 ACT periods cold or
                    # its weight-loads stall the whole in-order PE queue
                    for _ in range(3):
                        if pvq and pvq[0][0] <= gk - 2:
                            flush_one_half()
                    for t in grp["completes"]:
                        pvq.append((gk, h, t, tiles_meta, PT, OH, 0))
                        pvq.append((gk, h, t, tiles_meta, PT, OH, 1))
                        pvq.append((gk, h, t, tiles_meta, PT, OH, 2))
                    gk += 1
            while pvq:
                flush_one_half()

    nc.finalize()
    return nc


def _host_inputs(query, key, value):
    """Build the 8 per-core input maps (host-side sharding + layout prep)."""
    q = np.asarray(query, dtype=np.float32)
    k = np.asarray(key, dtype=np.float32)
    v = np.asarray(value, dtype=np.float32)

    pp = np.arange(128)[:, None]
    qq = np.arange(QTILE)[None, :]
    m1x = np.concatenate([(qq >= pp).astype(np.float32),
                          (qq[:, :128] >= pp).astype(np.float32)],
                         axis=1).astype(BF16)              # [128, 384]

    in_maps = []
    for c in range(NCORES):
        qTc = np.ascontiguousarray(q[:, c * 512:(c + 1) * 512].T).astype(BF16)
        kTc = np.ascontiguousarray(k[:, c * D:(c + 1) * D].T).astype(BF16)
        vc = v[:, c * D:(c + 1) * D]                         # [2048, 128]
        vch = vc.reshape(NCHUNK, 128, D).transpose(1, 0, 2)  # [128, 16, 128]
        v1c = np.ones((128, NCHUNK, 129), dtype=np.float32)
        v1c[:, :, :128] = vch
        v1c = v1c.reshape(128, NCHUNK * 129).astype(BF16)

        # staging: per (head-slot j, tile t>=4) up to 2 vertical-stride
        # blocks packed into one 128-row chunk; dead slots zero + masked
        ksc = np.zeros((128, HPC, 4, 128), dtype=np.float32)
        vsc = np.ones((128, HPC, 4, 129), dtype=np.float32)
        vsc[:, :, :, :128] = 0.0
        # pmask[..., 0:256]: leading staircase mask (pfx0 cols 0:64 =
        # blocks lo/lo+1 at q prefix 64; pfx1 cols 64:256 = blocks
        # lo+2/lo+3 at prefix 192), remote-selected blocks excluded
        # (staged instead). pmask[..., 256:512]: staging-slot selector
        # (per-partition, broadcast over the 256 staging columns).
        pmaskc = np.zeros((128, HPC, 4, 512), dtype=np.float32)
        for j in range(HPC):
            r = _remote_class(c, j)
            for t in range(4, 8):
                lo = 4 * t - 16
                for si, b in enumerate(_stage_blocks(c, j, t)):
                    ksc[:, j, t - 4, si * 64:(si + 1) * 64] = kTc[:, b * BLK:(b + 1) * BLK]
                    vsc[si * 64:(si + 1) * 64, j, t - 4, :128] = vch[
                        (b % 2) * 64:(b % 2) * 64 + 64, b // 2, :]
                    pmaskc[si * 64:(si + 1) * 64, j, t - 4, 256:512] = 1.0
                if (lo + 1) % VSTRIDE != r:
                    pmaskc[64:128, j, t - 4, 0:64] = 1.0
                if (lo + 2) % VSTRIDE != r:
                    pmaskc[0:64, j, t - 4, 64:192] = 1.0
                if (lo + 3) % VSTRIDE != r:
                    pmaskc[64:128, j, t - 4, 64:256] = 1.0

        in_maps.append({
            "qT": qTc,
            "kT": kTc,
            "v1": v1c,
            "m1x": m1x,
            "pmask": pmaskc.reshape(128, HPC * 4 * 512).astype(BF16),
            "ks": ksc.reshape(128, HPC * 4 * 128).astype(BF16),
            "vs": vsc.reshape(128, HPC * 4 * 129).astype(BF16),
        })
    return in_maps


def _get_nc():
    if "nc" not in _CACHE:
        _CACHE["nc"] = _build_nc()
    return _CACHE["nc"]


def kernel(query, key, value):
    from concourse.bass_utils import run_bass_kernel_spmd

    nc = _get_nc()
    in_maps = _host_inputs(query, key, value)
    res = run_bass_kernel_spmd(nc, in_maps, core_ids=list(range(NCORES)))
    outs = [res.results[c]["out"] for c in range(NCORES)]
    return np.concatenate(outs, axis=1).astype(np.float32)


if __name__ == "__main__":
    rng = np.random.default_rng(0)
    q = rng.standard_normal((S, NUM_HEADS * D), dtype=np.float32)
    k = rng.standard_normal((S, NUM_KV_HEADS * D), dtype=np.float32)
    v = rng.standard_normal((S, NUM_KV_HEADS * D), dtype=np.float32)
    o = kernel(query=q, key=k, value=v)
    print("kernel output", o.shape, o.dtype, np.abs(o).max())
